# revision 3
# baseline (speedup 1.0000x reference)
"""Trainium2 8-core causal single-head attention.

Problem: x[4,4096,768] @ Wq/Wk/Wv[768,64] -> causal softmax attention -> out[4,4096,64].

Sharding: 8 cores = 4 batches x 2 query-interleave groups. Core c handles
batch b=c//2, parity h=c%2: local q-tile i (16 tiles of 128 rows) maps to
global q-tile g=2i+h. Both cores of a batch compute full-context K/V
projections locally (no collectives).

Pair-swap trick: the host stores each core's xT with adjacent kv-tile pairs
swapped for h=1 cores, so in *position space* every core's own q tiles sit at
even positions and local q-tile i attends kv positions 0..2i+1 with position
2i getting a triangular mask and position 2i+1 a parity mask (zeros for h=0,
ones for h=1) - both pure data. One SPMD program serves both parities and q
is gathered straight out of the xkv tile with a strided access pattern (no
separate xq stream).

Host layout: xkv is chunk-major [8 chunks][128 rows][6 ec][512 cols] so each
512-col chunk is one contiguous 768KB DMA with 6KB/partition descriptors.

On-chip: projections contract E on partitions; scores are computed as
S^T[kv_p, q_f] = kT.T @ qT in a single <=1024-col matmul per (group, kv pair);
exp runs on the Scalar/ACT engine (its only job); diagonal masks multiply on
GpSimd; PV accumulates outT[d1, q] += v1[kv,65].T @ PT[kv, q] with v1=[v|ones]
so row 64 carries the softmax denominator (host divides + scatters).

Queues: input DMA alternates sync/gpsimd hardware queues in consume order;
scalar issues no DMA. Output drains per 512-col block as accumulation closes
(finer 256-col pieces at the very end to shorten the tail).
"""

import sys

sys.path.insert(0, "/opt/trn_rl_repo")

from contextlib import ExitStack

import numpy as np
import ml_dtypes

B, T, E, D = 4, 4096, 768, 64
P = 128
TQ = T // 2          # queries per core
NQT = TQ // P        # 16 local q tiles
NKV = T // P         # 32 kv tiles
EC = E // P          # 6 contraction chunks
NCH = 8              # xkv chunks of 512 cols
CHW = 512            # chunk width (cols)
BF16 = ml_dtypes.bfloat16
N_WARMUP = 7         # dummy matmuls covering the input-DMA landing window

_CACHE = {}


def _build_bass():
    import concourse.bacc as bacc
    import concourse.mybir as mybir
    import concourse.tile as tile

    nc = bacc.Bacc("TRN2", target_bir_lowering=False)
    f32 = mybir.dt.float32
    bf16 = mybir.dt.bfloat16

    # chunk-major xkv: row j*128+p holds chunk j, partition p, 6*512 cols
    xkv_d = nc.dram_tensor("xkv", (NCH * P, EC * CHW), bf16, kind="ExternalInput")
    wq_d = nc.dram_tensor("wq", (E, D), bf16, kind="ExternalInput")
    wkv_d = nc.dram_tensor("wkv", (E, 2 * D), bf16, kind="ExternalInput")
    mprev_d = nc.dram_tensor("mask_prev", (P, P), bf16, kind="ExternalInput")
    mlast_d = nc.dram_tensor("mask_last", (P, P), bf16, kind="ExternalInput")
    ident_d = nc.dram_tensor("ident", (P, D), bf16, kind="ExternalInput")
    out_d = nc.dram_tensor("out", (D + 1, TQ), f32, kind="ExternalOutput")

    with ExitStack() as ctx:
        tc = ctx.enter_context(tile.TileContext(nc))
        const = ctx.enter_context(tc.tile_pool(name="const", bufs=1))
        xpool = ctx.enter_context(tc.tile_pool(name="x", bufs=1))
        spool = ctx.enter_context(tc.tile_pool(name="sb", bufs=1))
        ptpool = ctx.enter_context(tc.tile_pool(name="pt", bufs=3))
        obpool = ctx.enter_context(tc.tile_pool(name="ob", bufs=2))
        # PSUM: sst 2x[128,1024]f32 = 4 banks, proj 2x[128,512]f32 = 2 banks,
        # outp [65,1024]f32 = 2 banks -> 8 banks exactly
        psst = ctx.enter_context(tc.tile_pool(name="psst", bufs=2, space="PSUM"))
        pprj = ctx.enter_context(tc.tile_pool(name="pprj", bufs=2, space="PSUM"))
        pout = ctx.enter_context(tc.tile_pool(name="pout", bufs=1, space="PSUM"))

        # ---- PE warmup: keep TensorE busy through the input-DMA landing so
        # the HAM clock gate ramps and the PE never idles before real work ----
        scratch = const.tile([P, 512], bf16)
        nc.vector.memset(scratch[:], 1.0)

        def warm(n):
            for wi in range(n):
                pw = psst.tile([P, 1024], f32, tag="ss", name=f"warm{wi}")
                nc.tensor.matmul(
                    pw[:, 0:512], lhsT=scratch[:, 0:P], rhs=scratch[:],
                    start=True, stop=True,
                )
                if wi == n - 1:
                    # consume the result so DCE keeps the warmup chain
                    nc.vector.tensor_copy(scratch[0:1, 0:1], pw[0:1, 0:1])

        warm(N_WARMUP)

        # ---- input DMA: weights/masks first (tiny, unblock first matmuls),
        # then xkv chunks in consume order alternating the two hw queues ----
        wq_t = const.tile([P, EC * D], bf16)
        nc.sync.dma_start(
            out=wq_t.rearrange("p (ec d) -> p ec d", d=D),
            in_=wq_d.rearrange("(ec p) d -> p ec d", p=P),
        )
        wkv_t = const.tile([P, EC * 2 * D], bf16)
        nc.sync.dma_start(
            out=wkv_t.rearrange("p (ec d) -> p ec d", d=2 * D),
            in_=wkv_d.rearrange("(ec p) d -> p ec d", p=P),
        )
        mprev_t = const.tile([P, P], bf16)
        nc.sync.dma_start(out=mprev_t[:], in_=mprev_d[:])
        mlast_t = const.tile([P, P], bf16)
        nc.sync.dma_start(out=mlast_t[:], in_=mlast_d[:])
        ident_t = const.tile([P, D], bf16)
        nc.sync.dma_start(out=ident_t[:], in_=ident_d[:])

        xkv_t = xpool.tile([P, NCH * EC * CHW], bf16)

        def dma_xkv(j, eng):
            eng.dma_start(
                out=xkv_t[:, j * EC * CHW:(j + 1) * EC * CHW],
                in_=xkv_d[j * P:(j + 1) * P, :],
            )

        for j in range(NCH):
            dma_xkv(j, nc.sync if j % 2 == 0 else nc.gpsimd)

        qT_t = spool.tile([D, TQ], bf16)
        kvT_t = spool.tile([P, T], bf16)
        v1_t = spool.tile([P, NKV * (D + 1)], bf16)
        nc.vector.memset(v1_t[:], 1.0)

        # strided q-gather view: chunk c, ec e, even positions (gh=0)
        xq_v = xkv_t.rearrange(
            "p (c e gp gh t) -> p gh c e gp t", c=NCH, e=EC, gp=2, gh=2, t=P
        )

        def qt_proj(jb):
            # local q cols [jb*512, (jb+1)*512) from chunks 2jb, 2jb+1
            ps = pprj.tile([P, 512], f32, tag="pj", name=f"psq{jb}")
            for ec in range(EC):
                nc.tensor.matmul(
                    ps[0:D, :],
                    lhsT=wq_t[:, ec * D:(ec + 1) * D],
                    rhs=xq_v[:, 0, 2 * jb:2 * jb + 2, ec, :, :],
                    start=(ec == 0),
                    stop=(ec == EC - 1),
                )
            nc.vector.tensor_copy(qT_t[:, jb * 512:(jb + 1) * 512], ps[0:D, :])

        def kv_proj_mm(j):
            # kT/vT columns j*512..(j+1)*512
            ps = pprj.tile([P, 512], f32, tag="pj", name=f"pskv{j}")
            for ec in range(EC):
                nc.tensor.matmul(
                    ps[:, :],
                    lhsT=wkv_t[:, ec * 2 * D:(ec + 1) * 2 * D],
                    rhs=xkv_t[:, j * EC * CHW + ec * CHW: j * EC * CHW + (ec + 1) * CHW],
                    start=(ec == 0),
                    stop=(ec == EC - 1),
                )
            nc.vector.tensor_copy(kvT_t[:, j * 512:(j + 1) * 512], ps[:, :])

        def v_transpose(j):
            # transpose the 4 v-tiles of chunk j into v1
            # (batched: 4 PE transposes -> one DVE copy)
            pv = pprj.tile([P, 512], bf16, tag="pj", name=f"psv{j}")
            for m in range(4):
                k = 4 * j + m
                nc.tensor.transpose(
                    pv[:, m * D:(m + 1) * D],
                    in_=kvT_t[D:2 * D, k * P:(k + 1) * P],
                    identity=ident_t[D:2 * D, :],
                )
            nc.vector.tensor_copy(
                v1_t.rearrange("p (k e) -> p k e", e=D + 1)[:, 4 * j:4 * j + 4, 0:D],
                pv.rearrange("p (m e) -> p m e", e=D)[:, 0:4, :],
            )

        outp_tiles = {}

        def attn_group(cq, ks):
            # q columns [cq*1024, (cq+1)*1024), kv position-pairs ks
            lo, hi = cq * 1024, (cq + 1) * 1024
            if cq not in outp_tiles:
                outp_tiles[cq] = pout.tile(
                    [D + 1, 1024], f32, tag="out", name=f"outp{cq}"
                )
            outp = outp_tiles[cq]
            pend = []

            def drain(c0, c1):
                ob = obpool.tile([D + 1, c1 - c0], f32)
                nc.vector.tensor_copy(ob[:], outp[:, c0 - lo: c1 - lo])
                nc.sync.dma_start(out=out_d[:, c0:c1], in_=ob[:])

            def flush_pv():
                k, pt, cs, w = pend.pop(0)
                v1k = v1_t[:, k * (D + 1):(k + 1) * (D + 1)]
                # unmasked halves first: the diagonal-mask multiply only
                # touches pt[:, 0:128] (half 0), so issuing later halves first
                # hides the mask latency under a 512-wide matmul
                for half in sorted(range(0, w, 512), reverse=True):
                    hw = min(512, w - half)
                    g512 = (cs + half) // 512
                    nc.tensor.matmul(
                        outp[:, cs + half - lo: cs + half - lo + hw],
                        lhsT=v1k,
                        rhs=pt[:, half:half + hw],
                        start=(k == 0),
                        stop=(k == 8 * g512 + 7),
                    )
                # drain each output block as soon as its accumulation closes;
                # the last block goes in two 256-col pieces to shorten the tail
                if k == 8 * (2 * cq) + 7:
                    drain(lo, lo + 512)
                elif k == 8 * (2 * cq + 1) + 7:
                    if cq == 1:
                        drain(1792, 2048)
                    else:
                        drain(lo + 512, lo + 1024)
                elif cq == 1 and k == 29:
                    drain(1536, 1792)

            for k in ks:
                qs = (k // 2) * P
                cs = max(qs, lo)
                w = hi - cs
                sst = psst.tile([P, 1024], f32, tag="ss", name=f"sst{cq}_{k}")
                for half in range(0, w, 512):
                    hw = min(512, w - half)
                    nc.tensor.matmul(
                        sst[:, half:half + hw],
                        lhsT=kvT_t[0:D, k * P:(k + 1) * P],
                        rhs=qT_t[:, cs + half: cs + half + hw],
                        start=True,
                        stop=True,
                    )
                pt = ptpool.tile([P, 1024], bf16)
                nc.scalar.activation(
                    pt[:, 0:w], sst[:, 0:w],
                    func=mybir.ActivationFunctionType.Exp, scale=0.125,
                )
                if cs == qs:
                    m = mprev_t if (k % 2 == 0) else mlast_t
                    nc.gpsimd.tensor_mul(pt[:, 0:P], pt[:, 0:P], m[:])
                pend.append((k, pt, cs, w))
                # scores of pair k+1 issue before PV of pair k: the PE never
                # sits through the Scalar-engine exp latency
                if len(pend) >= 2:
                    flush_pv()
            while pend:
                flush_pv()

        # deadline-ordered interleave: kv/q projections and transposes fill
        # the PE while the xkv stream lands; attention follows data arrival
        kv_proj_mm(0)
        v_transpose(0)
        qt_proj(0)
        kv_proj_mm(1)
        v_transpose(1)
        qt_proj(1)
        kv_proj_mm(2)
        v_transpose(2)
        attn_group(0, range(0, 4))
        kv_proj_mm(3)
        v_transpose(3)
        attn_group(0, range(4, 8))
        qt_proj(2)
        kv_proj_mm(4)
        v_transpose(4)
        attn_group(0, range(8, 12))
        kv_proj_mm(5)
        v_transpose(5)
        attn_group(0, range(12, 16))
        qt_proj(3)
        kv_proj_mm(6)
        v_transpose(6)
        attn_group(1, range(0, 8))
        kv_proj_mm(7)
        v_transpose(7)
        attn_group(1, range(8, 16))
        attn_group(1, range(16, 32))

    nc.compile()
    return nc


def _shard_inputs(x, Wq, Wk, Wv):
    x = np.asarray(x, np.float32)
    wqb = np.asarray(Wq, np.float32).astype(BF16)
    wkvb = np.concatenate([np.asarray(Wk, np.float32), np.asarray(Wv, np.float32)], axis=1).astype(BF16)
    ident = np.zeros((P, D), BF16)
    ident[D:2 * D, :] = np.eye(D, dtype=BF16)
    tri = (np.arange(P)[:, None] <= np.arange(P)[None, :]).astype(BF16)
    ones = np.ones((P, P), BF16)
    zeros = np.zeros((P, P), BF16)
    # pair-swap permutation for h=1 cores (1,0,3,2,...)
    perm_swap = np.arange(NKV) ^ 1
    in_maps = []
    xT_cache = {}
    for c in range(8):
        b, h = c // 2, c % 2
        if b not in xT_cache:
            xT_cache[b] = np.ascontiguousarray(x[b].T).astype(BF16)  # [768, 4096]
        xT = xT_cache[b]
        xt = xT.reshape(E, NKV, P)
        if h == 1:
            xt = xt[:, perm_swap, :]
        # chunk-major [8][128][6][512]
        xc = np.ascontiguousarray(
            xt.reshape(EC, P, NCH, CHW).transpose(2, 1, 0, 3)
        ).reshape(NCH * P, EC * CHW)
        in_maps.append({
            "xkv": xc,
            "wq": wqb,
            "wkv": wkvb,
            "mask_prev": tri,
            "mask_last": zeros if h == 0 else ones,
            "ident": ident,
        })
    return in_maps


def _unshard(results):
    out = np.zeros((B, T, D), np.float32)
    for c, om in enumerate(results):
        b, h = c // 2, c % 2
        o = np.asarray(om["out"], np.float32)               # [65, 2048]
        on = (o[:D] / o[D:D + 1]).T                         # [2048, 64]
        for i in range(NQT):
            out[b, (2 * i + h) * P:(2 * i + h + 1) * P] = on[i * P:(i + 1) * P]
    return out


def kernel(x, Wq, Wk, Wv):
    from concourse import bass_utils

    if "nc" not in _CACHE:
        _CACHE["nc"] = _build_bass()
    nc = _CACHE["nc"]
    in_maps = _shard_inputs(x, Wq, Wk, Wv)
    res = bass_utils.run_bass_kernel_spmd(nc, in_maps, core_ids=list(range(8)))
    _CACHE["last_result"] = res
    return _unshard(res.results)
